# revision 1
# baseline (speedup 1.0000x reference)
"""NT-Xent contrastive loss kernel for TRN2, 8 NeuronCores.

Problem: z_i, z_j [4096, 256] f32.
  reps = concat(l2norm(z_i), l2norm(z_j))      # [8192, 256]
  sim  = reps @ reps.T                          # [8192, 8192]
  sim  = where(eye, -1e9, sim) / 0.07
  positives[i] = sim[i, (i+4096) % 8192]
  loss = mean(-(positives - log(sum(exp(sim), axis=1))))

Sharding: rows of sim split across 8 cores (1024 rows each). Each core
receives the full rep matrix TRANSPOSED ([D=256, 8192], so the GEMM
operands need no on-device transpose) and ROTATED so its rows come
first (np.roll by -core*1024): one static SPMD program works for all
cores — local row r is global row core*1024+r, its self-match is at
local column r and its positive at local column r+4096.

Per core:
  1. Load raw zT as 8 SBUF tiles [128, 2048] (2 d-halves x 4 col groups).
  2. Column norms without any transpose: square on DVE, column-sums via
     a ones[128,128] matmul (broadcast across partitions), then
     inv = exp(-0.5*ln(ss)) on ACT -> inv_bcast [128, 2048] per group.
     Normalize columns: repsTn = zT * inv_bcast (DVE, fp32r rounded).
  3. Row norms come for free: the exp's per-partition scale.
     inv_tile[:, t] = diagonal of inv_bcast block t (eye-mask + reduce).
  4. Main loop over 8 row-tiles x 4 col-chunks [128, 2048]: fp32r
     matmuls of RAW rows x NORMALIZED cols into PSUM; zero the
     self-diagonal block (multiply by 1-I); extract the positive
     diagonal (multiply by I with fused row-reduce); one ACT Exp over
     the chunk with scale = inv_row/T (finishing row normalization) and
     fused row-sum accum.
  5. denom = sum(partials) - 1 (zeroed diag contributes exp(0)=1);
     per-row loss = ln(denom) - pos*inv_row/T; DMA out [128, 8].
Host sums the 8x1024 per-row losses and divides by 8192.
"""

import sys

import numpy as np

for _p in ("/opt/trn_rl_repo", "/root/.axon_site/_ro/trn_rl_repo"):
    if _p not in sys.path:
        sys.path.append(_p)

B = 4096
D = 256
N2 = 2 * B                  # 8192 total rows
NCORES = 8
ROWS_PER_CORE = N2 // NCORES    # 1024
RT = ROWS_PER_CORE // 128       # 8 row-tiles per core
CHUNK = 2048                    # col-chunk (4 PSUM banks)
NCHUNK = N2 // CHUNK            # 4
PSUM_BUFS = 2
TEMP = 0.07
INV_T = 1.0 / TEMP

_CACHE = {}


def _steer_act_tables():
    """Make Exp and Ln resolve to the one table set that holds both.

    bacc's insert_act_table_loads otherwise picks exp_and_others for Exp
    and natural_log for Ln, inserting a ~1.3us ACT table reload at every
    Ln<->Exp transition (~9 reloads here). Removing Exp/Ln from every
    other set in the map it consults (set contents only — list order and
    therefore act_func_set_id stay intact) pins both to
    natural_log_exp_and_others, so exactly one load is emitted.
    """
    import concourse.bacc as bacc
    import concourse.mybir as mybir

    if getattr(_steer_act_tables, "done", False):
        return
    A = mybir.ActivationFunctionType
    orig = bacc.get_activation_tables

    def patched(arch):
        tabs = orig(arch)
        for name, funcs in tabs.items():
            if name != "natural_log_exp_and_others":
                funcs.discard(A.Exp)
                funcs.discard(A.Ln)
        return tabs

    bacc.get_activation_tables = patched
    _steer_act_tables.done = True


def _build_nc(ablate=()):
    import concourse.bacc as bacc
    import concourse.mybir as mybir
    import concourse.tile as tile
    from contextlib import ExitStack

    _steer_act_tables()

    f32 = mybir.dt.float32
    f32r = mybir.dt.float32r
    Act = mybir.ActivationFunctionType
    Alu = mybir.AluOpType
    Ax = mybir.AxisListType

    nc = bacc.Bacc("TRN2", target_bir_lowering=False, debug=False)

    repsT_d = nc.dram_tensor("repsT", [D, N2], f32r, kind="ExternalInput").ap()
    masks_d = nc.dram_tensor("masks", [128, 256], f32, kind="ExternalInput").ap()
    ones_d = nc.dram_tensor("ones", [128, 128], f32r, kind="ExternalInput").ap()
    lout_d = nc.dram_tensor("lout", [128, RT], f32, kind="ExternalOutput").ap()

    with tile.TileContext(nc) as tc, ExitStack() as ctx:
        const_pool = ctx.enter_context(tc.tile_pool(name="const", bufs=1))
        zt_pool = ctx.enter_context(tc.tile_pool(name="zt", bufs=1))
        zsq_pool = ctx.enter_context(tc.tile_pool(name="zsq", bufs=3))
        norm_pool = ctx.enter_context(tc.tile_pool(name="norm", bufs=2))
        rn_pool = ctx.enter_context(tc.tile_pool(name="rn", bufs=1))
        small_pool = ctx.enter_context(tc.tile_pool(name="small", bufs=1))
        psum_pool = ctx.enter_context(tc.tile_pool(name="psum", bufs=PSUM_BUFS, space="PSUM"))

        masks_sb = const_pool.tile([128, 256], f32, tag="masks", name="masks")
        nc.sync.dma_start(masks_sb[:], masks_d)
        eye_sb = masks_sb[:, 0:128]
        ome_sb = masks_sb[:, 128:256]
        ones_sb = const_pool.tile([128, 128], f32r, tag="ones", name="ones")
        nc.sync.dma_start(ones_sb[:], ones_d)

        # raw zT tiles [d-half h][col group g]: [128, 2048]
        zT = [
            [zt_pool.tile([128, CHUNK], f32r, tag=f"zT{h}_{g}", name=f"zT{h}_{g}")
             for g in range(NCHUNK)]
            for h in range(2)
        ]
        # normalized columns
        repsTn = [
            [rn_pool.tile([128, CHUNK], f32r, tag=f"rn{h}_{g}", name=f"rn{h}_{g}")
             for g in range(NCHUNK)]
            for h in range(2)
        ]

        partials = small_pool.tile([128, RT * NCHUNK], f32, tag="partials", name="partials")
        pos_all = small_pool.tile([128, RT], f32, tag="pos", name="pos")
        den_all = small_pool.tile([128, RT], f32, tag="den", name="den")
        logden = small_pool.tile([128, RT], f32, tag="logden", name="logden")
        loss_t = small_pool.tile([128, RT], f32, tag="loss", name="loss")
        inv_tile = small_pool.tile([128, RT], f32, tag="invt", name="invt")
        inv_tile_T = small_pool.tile([128, RT], f32, tag="invtT", name="invtT")
        pos_sc = small_pool.tile([128, RT], f32, tag="possc", name="possc")
        junk128 = small_pool.tile([128, 128], f32, tag="junk128", name="junk128")

        def load_group(g):
            for h in range(2):
                for q in range(2):
                    sl = slice(q * 1024, (q + 1) * 1024)
                    nc.sync.dma_start(
                        zT[h][g][:, sl],
                        repsT_d[h * 128:(h + 1) * 128,
                                g * CHUNK + q * 1024:g * CHUNK + (q + 1) * 1024],
                    )

        def prologue_group(g):
            """Compute col norms for group g, normalize."""
            if "nonorm" in ablate:
                return
            sq_eng = nc.vector
            zsq = [None, None]
            for h in range(2):
                zq = zsq_pool.tile([128, CHUNK], f32r, tag="zsq", name="zsq")
                for q in range(4):
                    sl = slice(q * 512, (q + 1) * 512)
                    sq_eng.tensor_mul(
                        zq[:, sl], zT[h][g][:, sl], zT[h][g][:, sl]
                    )
                zsq[h] = zq
            ssp = psum_pool.tile([128, CHUNK], f32, tag="ps", name="ps")
            for h in range(2):
                for b in range(CHUNK // 512):
                    nc.tensor.matmul(
                        ssp[:, b * 512:(b + 1) * 512],
                        ones_sb[:],
                        zsq[h][:, b * 512:(b + 1) * 512],
                        start=(h == 0),
                        stop=(h == 1),
                    )
            # inv = ss^-0.5 broadcast over partitions; ln psum->sbuf, exp in place
            inv_b = norm_pool.tile([128, CHUNK], f32, tag="invb", name="invb")
            nc.scalar.activation(inv_b[:], ssp[:], Act.Ln)
            nc.scalar.activation(inv_b[:], inv_b[:], Act.Exp, scale=-0.5)
            # GPSIMD is otherwise idle; putting the column-normalize
            # multiplies there unclogs DVE's queue ahead of the main loop
            scale_eng = nc.gpsimd
            for h in range(2):
                for q in range(4):
                    sl = slice(q * 512, (q + 1) * 512)
                    scale_eng.tensor_mul(
                        repsTn[h][g][:, sl], zT[h][g][:, sl], inv_b[:, sl]
                    )
            if g == 0:
                # per-row inv for my rows: diagonal of each 128-block
                for t in range(RT):
                    nc.vector.scalar_tensor_tensor(
                        out=junk128[:],
                        in0=inv_b[:, t * 128:(t + 1) * 128], scalar=1.0,
                        in1=eye_sb, op0=Alu.mult, op1=Alu.mult,
                        accum_out=inv_tile[:, t:t + 1],
                    )
                nc.vector.tensor_scalar_mul(inv_tile_T[:], inv_tile[:], INV_T)

        def main_chunk(c, ts=None):
            """sim rows (row-tiles ts) x cols [c*2048, (c+1)*2048)."""
            for t in (range(RT) if ts is None else ts):
                ps = psum_pool.tile([128, CHUNK], f32, tag="ps", name="ps")
                for h in range(2):
                    lhsT = zT[h][0][:, t * 128:(t + 1) * 128]
                    for b in range(CHUNK // 512):
                        nc.tensor.matmul(
                            ps[:, b * 512:(b + 1) * 512],
                            lhsT,
                            repsTn[h][c][:, b * 512:(b + 1) * 512],
                            start=(h == 0),
                            stop=(h == 1),
                        )
                if c == 0:
                    # zero the self-similarity diagonal (block at cols t*128)
                    nc.vector.tensor_mul(
                        ps[:, t * 128:(t + 1) * 128],
                        ps[:, t * 128:(t + 1) * 128],
                        ome_sb,
                    )
                if c == 2:
                    # positive-pair diagonal, raw dot (row scale applied later)
                    nc.vector.scalar_tensor_tensor(
                        out=junk128[:],
                        in0=ps[:, t * 128:(t + 1) * 128], scalar=1.0,
                        in1=eye_sb, op0=Alu.mult, op1=Alu.mult,
                        accum_out=pos_all[:, t:t + 1],
                    )
                if "noexp" in ablate:
                    continue
                # exp(sim * inv_row / T) in place + fused row-sum; the
                # per-partition scale finishes the row normalization
                nc.scalar.activation(
                    ps[:], ps[:], Act.Exp, scale=inv_tile_T[:, t:t + 1],
                    accum_out=partials[:, t * NCHUNK + c:t * NCHUNK + c + 1],
                )

        def warmup(n):
            # Dummy fp32 matmuls (masks@masks) to keep the PE HAM ramp hot
            # through load-gated gaps; they recycle a main psum slot that
            # is cleared by the next start=True accumulation.
            wps = psum_pool.tile([128, CHUNK], f32, tag="ps", name="ps")
            for _ in range(n):
                nc.tensor.matmul(
                    wps[:, 0:256], masks_sb[:, 0:128], masks_sb[:],
                    start=True, stop=True,
                )

        import os as _os2
        order = _os2.environ.get("K_ORDER", "ahead1")
        WARM1 = int(_os2.environ.get("K_WARM1", "0"))
        WARM2 = int(_os2.environ.get("K_WARM2", "0"))
        if WARM1:
            warmup(WARM1)
        for g in range(NCHUNK):
            load_group(g)
        LEAD = int(_os2.environ.get("K_LEAD", "0"))
        if order == "ahead1":
            prologue_group(0)
            if WARM2:
                warmup(WARM2)
            for g in range(NCHUNK):
                if "nomm" in ablate:
                    if g + 1 < NCHUNK:
                        prologue_group(g + 1)
                    continue
                main_chunk(g, range(0, LEAD))
                if g + 1 < NCHUNK:
                    prologue_group(g + 1)
                main_chunk(g, range(LEAD, RT))
        elif order == "ahead_half":
            prologue_group(0)
            for g in range(NCHUNK):
                if "nomm" in ablate:
                    if g + 1 < NCHUNK:
                        prologue_group(g + 1)
                    continue
                main_chunk(g, range(0, RT // 2))
                if g + 1 < NCHUNK:
                    prologue_group(g + 1)
                main_chunk(g, range(RT // 2, RT))
        elif order == "interleave":
            for g in range(NCHUNK):
                prologue_group(g)
                if "nomm" not in ablate:
                    main_chunk(g)
        elif order == "prologue_first":
            for g in range(NCHUNK):
                prologue_group(g)
            for g in range(NCHUNK):
                if "nomm" not in ablate:
                    main_chunk(g)

        if ablate:
            nc.vector.memset(loss_t[:], 1.0)
        else:
            # denom = sum_c partials - 1 (zeroed diag contributed exp(0)=1)
            nc.vector.reduce_sum(
                den_all[:], partials[:].rearrange("p (t c) -> p t c", c=NCHUNK),
                axis=Ax.X,
            )
            nc.vector.tensor_scalar_add(den_all[:], den_all[:], -1.0)
            nc.scalar.activation(logden[:], den_all[:], Act.Ln)
            # loss = ln(denom) - pos * inv_row / T
            nc.vector.tensor_mul(pos_sc[:], pos_all[:], inv_tile_T[:])
            nc.vector.tensor_sub(loss_t[:], logden[:], pos_sc[:])
        nc.sync.dma_start(lout_d, loss_t[:])

    nc.finalize()
    return nc


def _get_nc():
    if "nc" not in _CACHE:
        _CACHE["nc"] = _build_nc()
    return _CACHE["nc"]


def _in_maps(z_i, z_j):
    reps = np.concatenate(
        [np.asarray(z_i, np.float32), np.asarray(z_j, np.float32)], axis=0
    )
    eye = np.eye(128, dtype=np.float32)
    masks = np.concatenate([eye, 1.0 - eye], axis=1).astype(np.float32)
    ones128 = np.ones((128, 128), dtype=np.float32)
    maps = []
    for m in range(NCORES):
        rotT = np.ascontiguousarray(np.roll(reps, -m * ROWS_PER_CORE, axis=0).T)
        maps.append({"repsT": rotT, "masks": masks, "ones": ones128})
    return maps


def kernel(z_i, z_j):
    from concourse.bass_utils import run_bass_kernel_spmd

    nc = _get_nc()
    res = run_bass_kernel_spmd(nc, _in_maps(z_i, z_j), list(range(NCORES)))
    total = 0.0
    for r in res.results:
        total += float(np.sum(r["lout"], dtype=np.float64))
    return np.float32(total / N2)



# revision 3
# speedup vs baseline: 1.0152x; 1.0152x over previous
"""NT-Xent contrastive loss kernel for TRN2, 8 NeuronCores.

Problem: z_i, z_j [4096, 256] f32.
  reps = concat(l2norm(z_i), l2norm(z_j))      # [8192, 256]
  sim  = reps @ reps.T                          # [8192, 8192]
  sim  = where(eye, -1e9, sim) / 0.07
  positives[i] = sim[i, (i+4096) % 8192]
  loss = mean(-(positives - log(sum(exp(sim), axis=1))))

Sharding: rows of sim split across 8 cores (1024 rows each). Each core
receives the full rep matrix TRANSPOSED ([D=256, 8192], so the GEMM
operands need no on-device transpose) and ROTATED so its rows come
first (np.roll by -core*1024): one static SPMD program works for all
cores — local row r is global row core*1024+r, its self-match is at
local column r and its positive at local column r+4096.

Per core:
  1. Load raw zT as 8 SBUF tiles [128, 2048] (2 d-halves x 4 col groups).
  2. Column norms without any transpose: square on DVE, column-sums via
     a ones[128,128] matmul (broadcast across partitions), then
     inv = exp(-0.5*ln(ss)) on ACT -> inv_bcast [128, 2048] per group.
     Normalize columns: repsTn = zT * inv_bcast (DVE, fp32r rounded).
  3. Row norms come for free: the exp's per-partition scale.
     inv_tile[:, t] = diagonal of inv_bcast block t (eye-mask + reduce).
  4. Main loop over 8 row-tiles x 4 col-chunks [128, 2048]: fp32r
     matmuls of RAW rows x NORMALIZED cols into PSUM; zero the
     self-diagonal block (multiply by 1-I); extract the positive
     diagonal (multiply by I with fused row-reduce); one ACT Exp over
     the chunk with scale = inv_row/T (finishing row normalization) and
     fused row-sum accum.
  5. denom = sum(partials) - 1 (zeroed diag contributes exp(0)=1);
     per-row loss = ln(denom) - pos*inv_row/T; DMA out [128, 8].
Host sums the 8x1024 per-row losses and divides by 8192.
"""

import sys

import numpy as np

for _p in ("/opt/trn_rl_repo", "/root/.axon_site/_ro/trn_rl_repo"):
    if _p not in sys.path:
        sys.path.append(_p)

B = 4096
D = 256
N2 = 2 * B                  # 8192 total rows
NCORES = 8
ROWS_PER_CORE = N2 // NCORES    # 1024
RT = ROWS_PER_CORE // 128       # 8 row-tiles per core
CHUNK = 2048                    # col-chunk (4 PSUM banks)
NCHUNK = N2 // CHUNK            # 4
PSUM_BUFS = 2
TEMP = 0.07
INV_T = 1.0 / TEMP

_CACHE = {}


def _steer_act_tables():
    """Make Exp and Ln resolve to the one table set that holds both.

    bacc's insert_act_table_loads otherwise picks exp_and_others for Exp
    and natural_log for Ln, inserting a ~1.3us ACT table reload at every
    Ln<->Exp transition (~9 reloads here). Removing Exp/Ln from every
    other set in the map it consults (set contents only — list order and
    therefore act_func_set_id stay intact) pins both to
    natural_log_exp_and_others, so exactly one load is emitted.
    """
    import concourse.bacc as bacc
    import concourse.mybir as mybir

    if getattr(_steer_act_tables, "done", False):
        return
    A = mybir.ActivationFunctionType
    orig = bacc.get_activation_tables

    def patched(arch):
        tabs = orig(arch)
        for name, funcs in tabs.items():
            if name != "natural_log_exp_and_others":
                funcs.discard(A.Exp)
                funcs.discard(A.Ln)
        return tabs

    bacc.get_activation_tables = patched
    _steer_act_tables.done = True


def _build_nc(ablate=()):
    import concourse.bacc as bacc
    import concourse.mybir as mybir
    import concourse.tile as tile
    from contextlib import ExitStack

    _steer_act_tables()

    f32 = mybir.dt.float32
    f32r = mybir.dt.float32r
    Act = mybir.ActivationFunctionType
    Alu = mybir.AluOpType
    Ax = mybir.AxisListType

    nc = bacc.Bacc("TRN2", target_bir_lowering=False, debug=False)

    repsT_d = nc.dram_tensor("repsT", [D, N2], f32r, kind="ExternalInput").ap()
    masks_d = nc.dram_tensor("masks", [128, 256], f32, kind="ExternalInput").ap()
    ones_d = nc.dram_tensor("ones", [128, 128], f32r, kind="ExternalInput").ap()
    lout_d = nc.dram_tensor("lout", [128, RT], f32, kind="ExternalOutput").ap()

    with tile.TileContext(nc) as tc, ExitStack() as ctx:
        const_pool = ctx.enter_context(tc.tile_pool(name="const", bufs=1))
        zt_pool = ctx.enter_context(tc.tile_pool(name="zt", bufs=1))
        zsq_pool = ctx.enter_context(tc.tile_pool(name="zsq", bufs=3))
        norm_pool = ctx.enter_context(tc.tile_pool(name="norm", bufs=2))
        rn_pool = ctx.enter_context(tc.tile_pool(name="rn", bufs=1))
        small_pool = ctx.enter_context(tc.tile_pool(name="small", bufs=1))
        psum_pool = ctx.enter_context(tc.tile_pool(name="psum", bufs=PSUM_BUFS, space="PSUM"))

        masks_sb = const_pool.tile([128, 256], f32, tag="masks", name="masks")
        nc.sync.dma_start(masks_sb[:], masks_d)
        eye_sb = masks_sb[:, 0:128]
        ome_sb = masks_sb[:, 128:256]
        ones_sb = const_pool.tile([128, 128], f32r, tag="ones", name="ones")
        nc.sync.dma_start(ones_sb[:], ones_d)

        # raw zT tiles [d-half h][col group g]: [128, 2048]
        zT = [
            [zt_pool.tile([128, CHUNK], f32r, tag=f"zT{h}_{g}", name=f"zT{h}_{g}")
             for g in range(NCHUNK)]
            for h in range(2)
        ]
        # normalized columns
        repsTn = [
            [rn_pool.tile([128, CHUNK], f32r, tag=f"rn{h}_{g}", name=f"rn{h}_{g}")
             for g in range(NCHUNK)]
            for h in range(2)
        ]

        partials = small_pool.tile([128, RT * NCHUNK], f32, tag="partials", name="partials")
        pos_all = small_pool.tile([128, RT], f32, tag="pos", name="pos")
        den_all = small_pool.tile([128, RT], f32, tag="den", name="den")
        logden = small_pool.tile([128, RT], f32, tag="logden", name="logden")
        loss_t = small_pool.tile([128, RT], f32, tag="loss", name="loss")
        inv_tile = small_pool.tile([128, RT], f32, tag="invt", name="invt")
        inv_tile_T = small_pool.tile([128, RT], f32, tag="invtT", name="invtT")
        pos_sc = small_pool.tile([128, RT], f32, tag="possc", name="possc")
        junk128 = small_pool.tile([128, 128], f32, tag="junk128", name="junk128")

        def load_group(g):
            for h in range(2):
                for q in range(2):
                    sl = slice(q * 1024, (q + 1) * 1024)
                    nc.sync.dma_start(
                        zT[h][g][:, sl],
                        repsT_d[h * 128:(h + 1) * 128,
                                g * CHUNK + q * 1024:g * CHUNK + (q + 1) * 1024],
                    )

        def prologue_group_fine(g):
            """Group-0 prologue, pipelined per 512-col slice so the main
            loop's first matmul+exp start ~10us earlier: each slice's
            square -> ones-mm -> Ln -> Exp -> normalize chain only waits
            on its own 512 columns, not the whole 2048."""
            if "nonorm" in ablate:
                return
            zsq = [zsq_pool.tile([128, CHUNK], f32r, tag="zsq", name="zsq")
                   for _ in range(2)]
            ssp = psum_pool.tile([128, CHUNK], f32, tag="ps", name="ps")
            inv_b = norm_pool.tile([128, CHUNK], f32, tag="invb", name="invb")
            for q in range(4):
                sl = slice(q * 512, (q + 1) * 512)
                for h in range(2):
                    nc.vector.tensor_mul(
                        zsq[h][:, sl], zT[h][g][:, sl], zT[h][g][:, sl]
                    )
                for h in range(2):
                    nc.tensor.matmul(
                        ssp[:, sl], ones_sb[:], zsq[h][:, sl],
                        start=(h == 0), stop=(h == 1),
                    )
                nc.scalar.activation(inv_b[:, sl], ssp[:, sl], Act.Ln)
                nc.scalar.activation(inv_b[:, sl], inv_b[:, sl], Act.Exp,
                                     scale=-0.5)
                for h in range(2):
                    nc.gpsimd.tensor_mul(
                        repsTn[h][g][:, sl], zT[h][g][:, sl], inv_b[:, sl]
                    )
                if q < 2:
                    # per-row inv for my rows lives in cols 0..1024
                    for t in range(4 * q, 4 * q + 4):
                        nc.vector.scalar_tensor_tensor(
                            out=junk128[:],
                            in0=inv_b[:, t * 128:(t + 1) * 128], scalar=1.0,
                            in1=eye_sb, op0=Alu.mult, op1=Alu.mult,
                            accum_out=inv_tile[:, t:t + 1],
                        )
            nc.vector.tensor_scalar_mul(inv_tile_T[:], inv_tile[:], INV_T)

        def prologue_group(g):
            """Compute col norms for group g, normalize."""
            if "nonorm" in ablate:
                return
            sq_eng = nc.vector
            zsq = [None, None]
            for h in range(2):
                zq = zsq_pool.tile([128, CHUNK], f32r, tag="zsq", name="zsq")
                for q in range(4):
                    sl = slice(q * 512, (q + 1) * 512)
                    sq_eng.tensor_mul(
                        zq[:, sl], zT[h][g][:, sl], zT[h][g][:, sl]
                    )
                zsq[h] = zq
            ssp = psum_pool.tile([128, CHUNK], f32, tag="ps", name="ps")
            for h in range(2):
                for b in range(CHUNK // 512):
                    nc.tensor.matmul(
                        ssp[:, b * 512:(b + 1) * 512],
                        ones_sb[:],
                        zsq[h][:, b * 512:(b + 1) * 512],
                        start=(h == 0),
                        stop=(h == 1),
                    )
            # inv = ss^-0.5 broadcast over partitions; ln psum->sbuf, exp in place
            inv_b = norm_pool.tile([128, CHUNK], f32, tag="invb", name="invb")
            nc.scalar.activation(inv_b[:], ssp[:], Act.Ln)
            nc.scalar.activation(inv_b[:], inv_b[:], Act.Exp, scale=-0.5)
            # GPSIMD is otherwise idle; putting the column-normalize
            # multiplies there unclogs DVE's queue ahead of the main loop
            scale_eng = nc.gpsimd
            for h in range(2):
                for q in range(4):
                    sl = slice(q * 512, (q + 1) * 512)
                    scale_eng.tensor_mul(
                        repsTn[h][g][:, sl], zT[h][g][:, sl], inv_b[:, sl]
                    )
            if g == 0:
                # per-row inv for my rows: diagonal of each 128-block
                for t in range(RT):
                    nc.vector.scalar_tensor_tensor(
                        out=junk128[:],
                        in0=inv_b[:, t * 128:(t + 1) * 128], scalar=1.0,
                        in1=eye_sb, op0=Alu.mult, op1=Alu.mult,
                        accum_out=inv_tile[:, t:t + 1],
                    )
                nc.vector.tensor_scalar_mul(inv_tile_T[:], inv_tile[:], INV_T)

        def main_chunk(c, ts=None):
            """sim rows (row-tiles ts) x cols [c*2048, (c+1)*2048)."""
            for t in (range(RT) if ts is None else ts):
                ps = psum_pool.tile([128, CHUNK], f32, tag="ps", name="ps")
                for h in range(2):
                    lhsT = zT[h][0][:, t * 128:(t + 1) * 128]
                    for b in range(CHUNK // 512):
                        nc.tensor.matmul(
                            ps[:, b * 512:(b + 1) * 512],
                            lhsT,
                            repsTn[h][c][:, b * 512:(b + 1) * 512],
                            start=(h == 0),
                            stop=(h == 1),
                        )
                if c == 0:
                    # zero the self-similarity diagonal (block at cols t*128)
                    nc.vector.tensor_mul(
                        ps[:, t * 128:(t + 1) * 128],
                        ps[:, t * 128:(t + 1) * 128],
                        ome_sb,
                    )
                if c == 2:
                    # positive-pair diagonal, raw dot (row scale applied later)
                    nc.vector.scalar_tensor_tensor(
                        out=junk128[:],
                        in0=ps[:, t * 128:(t + 1) * 128], scalar=1.0,
                        in1=eye_sb, op0=Alu.mult, op1=Alu.mult,
                        accum_out=pos_all[:, t:t + 1],
                    )
                if "noexp" in ablate:
                    continue
                # exp(sim * inv_row / T) in place + fused row-sum; the
                # per-partition scale finishes the row normalization
                nc.scalar.activation(
                    ps[:], ps[:], Act.Exp, scale=inv_tile_T[:, t:t + 1],
                    accum_out=partials[:, t * NCHUNK + c:t * NCHUNK + c + 1],
                )

        def warmup(n):
            # Dummy fp32 matmuls (masks@masks) to keep the PE HAM ramp hot
            # through load-gated gaps; they recycle a main psum slot that
            # is cleared by the next start=True accumulation.
            wps = psum_pool.tile([128, CHUNK], f32, tag="ps", name="ps")
            for _ in range(n):
                nc.tensor.matmul(
                    wps[:, 0:256], masks_sb[:, 0:128], masks_sb[:],
                    start=True, stop=True,
                )

        import os as _os2
        order = _os2.environ.get("K_ORDER", "ahead1")
        WARM1 = int(_os2.environ.get("K_WARM1", "0"))
        WARM2 = int(_os2.environ.get("K_WARM2", "0"))
        if WARM1:
            warmup(WARM1)
        for g in range(NCHUNK):
            load_group(g)
        LEAD = int(_os2.environ.get("K_LEAD", "0"))
        if order == "ahead1":
            prologue_group_fine(0)
            if WARM2:
                warmup(WARM2)
            for g in range(NCHUNK):
                if "nomm" in ablate:
                    if g + 1 < NCHUNK:
                        prologue_group(g + 1)
                    continue
                main_chunk(g, range(0, LEAD))
                if g + 1 < NCHUNK:
                    prologue_group(g + 1)
                main_chunk(g, range(LEAD, RT))
        elif order == "ahead_half":
            prologue_group_fine(0)
            for g in range(NCHUNK):
                if "nomm" in ablate:
                    if g + 1 < NCHUNK:
                        prologue_group(g + 1)
                    continue
                main_chunk(g, range(0, RT // 2))
                if g + 1 < NCHUNK:
                    prologue_group(g + 1)
                main_chunk(g, range(RT // 2, RT))
        elif order == "interleave":
            for g in range(NCHUNK):
                prologue_group(g)
                if "nomm" not in ablate:
                    main_chunk(g)
        elif order == "prologue_first":
            for g in range(NCHUNK):
                prologue_group(g)
            for g in range(NCHUNK):
                if "nomm" not in ablate:
                    main_chunk(g)

        if ablate:
            nc.vector.memset(loss_t[:], 1.0)
        else:
            # denom = sum_c partials - 1 (zeroed diag contributed exp(0)=1)
            nc.vector.reduce_sum(
                den_all[:], partials[:].rearrange("p (t c) -> p t c", c=NCHUNK),
                axis=Ax.X,
            )
            nc.vector.tensor_scalar_add(den_all[:], den_all[:], -1.0)
            nc.scalar.activation(logden[:], den_all[:], Act.Ln)
            # loss = ln(denom) - pos * inv_row / T
            nc.vector.tensor_mul(pos_sc[:], pos_all[:], inv_tile_T[:])
            nc.vector.tensor_sub(loss_t[:], logden[:], pos_sc[:])
        nc.sync.dma_start(lout_d, loss_t[:])

    nc.finalize()
    return nc


def _get_nc():
    if "nc" not in _CACHE:
        _CACHE["nc"] = _build_nc()
    return _CACHE["nc"]


def _in_maps(z_i, z_j):
    reps = np.concatenate(
        [np.asarray(z_i, np.float32), np.asarray(z_j, np.float32)], axis=0
    )
    eye = np.eye(128, dtype=np.float32)
    masks = np.concatenate([eye, 1.0 - eye], axis=1).astype(np.float32)
    ones128 = np.ones((128, 128), dtype=np.float32)
    maps = []
    for m in range(NCORES):
        rotT = np.ascontiguousarray(np.roll(reps, -m * ROWS_PER_CORE, axis=0).T)
        maps.append({"repsT": rotT, "masks": masks, "ones": ones128})
    return maps


def kernel(z_i, z_j):
    from concourse.bass_utils import run_bass_kernel_spmd

    nc = _get_nc()
    res = run_bass_kernel_spmd(nc, _in_maps(z_i, z_j), list(range(NCORES)))
    total = 0.0
    for r in res.results:
        total += float(np.sum(r["lout"], dtype=np.float64))
    return np.float32(total / N2)

